# revision 11
# baseline (speedup 1.0000x reference)
"""Trainium2 Bass kernel for nn_MultiHeadHAR (2-layer LSTM encoder + 2-layer
LSTM decoder + heads), data-parallel over batch across 8 NeuronCores.

Self-contained: hardcodes shapes, packs weights on host into SBUF images,
builds one SPMD Bass program, runs it via run_bass_kernel_spmd, and
reassembles the full outputs.
"""

import numpy as np

import concourse.bacc as bacc
import concourse.bass as bass
import concourse.mybir as mybir
import concourse.tile as tile
from concourse.masks import make_identity
from concourse.bass_utils import run_bass_kernel_spmd

F32 = mybir.dt.float32
F32R = mybir.dt.float32r
AF = mybir.ActivationFunctionType

B, T, D, H, NCLS, PS = 128, 256, 6, 512, 4, 12
NCORES = 8
BL = B // NCORES          # 16 batch rows per core
G = 4 * H                 # 2048 gate width
KH = H // 128             # 4 partition chunks per hidden vector

# matmul input dtype (float32r streams 1 col/cycle vs 4 for float32)
MM_DT = F32R


def _mm(nc, out, lhsT, rhs, start, stop):
    nc.tensor.matmul(out, lhsT, rhs, start=start, stop=stop)


class _Cell:
    """Emits one LSTM cell step given transposed-state lhsT chunks."""

    def __init__(self, nc, pools, ident):
        self.nc = nc
        self.pools = pools
        self.ident = ident

    def gates(self, lhsT_chunks, w_img, tag):
        """lhsT_chunks: list of ([K,16] AP).  w_img: sbuf [128, nk*2048] AP
        (chunk k at cols [2048k:2048(k+1)]).  Returns 4 psum gate tiles."""
        nc, pg = self.nc, self.pools["pg"]
        gps = []
        for n in range(4):
            gp = pg.tile([BL, 512], F32, tag="g")
            nk = len(lhsT_chunks)
            for k, lhsT in enumerate(lhsT_chunks):
                rhs = w_img[k][:, 512 * n : 512 * (n + 1)]
                _mm(nc, gp[:], lhsT, rhs, start=(k == 0), stop=(k == nk - 1))
            gps.append(gp)
        return gps

    def elementwise(self, gps, c_prev, bias, tag):
        """gps: psum gate tiles [i,f,g,o]; bias: None or sbuf [16,2048] AP.
        Returns (h [16,512] sbuf, c_new [16,512] sbuf)."""
        nc, ps = self.nc, self.pools["ps"]
        acts = []
        for n, fn in enumerate([AF.Sigmoid, AF.Sigmoid, AF.Tanh, AF.Sigmoid]):
            a = ps.tile([BL, 512], F32, tag=f"a{n}_{tag}")
            if bias is not None:
                nc.vector.tensor_add(
                    gps[n][:], gps[n][:], bias[:, 512 * n : 512 * (n + 1)]
                )
            nc.scalar.activation(a[:], gps[n][:], fn)
            acts.append(a)
        i_s, f_s, g_t, o_s = acts
        # i_s <- i_s * tanh(g);  f_s <- f_s * c_prev;  c_new = i_s + f_s
        nc.vector.tensor_mul(i_s[:], i_s[:], g_t[:])
        nc.vector.tensor_mul(f_s[:], c_prev[:], f_s[:])
        c_new = ps.tile([BL, 512], F32, tag=f"c_{tag}")
        nc.vector.tensor_add(c_new[:], f_s[:], i_s[:])
        # g_t <- tanh(c_new);  h (in f_s slot) = o_s * g_t
        nc.scalar.activation(g_t[:], c_new[:], AF.Tanh)
        nc.vector.tensor_mul(f_s[:], o_s[:], g_t[:])
        return f_s, c_new

    def transpose(self, h, tag):
        """h: [16,512] sbuf -> hT sbuf tile [128, 64] (chunk q at cols 16q)."""
        nc = self.nc
        pt, ps = self.pools["pt"], self.pools["ps"]
        hT = ps.tile([128, KH * BL], MM_DT, tag=f"hT_{tag}")
        for q in range(KH):
            tp = pt.tile([128, BL], F32, tag="tp")
            nc.tensor.transpose(
                tp[:], h[:, 128 * q : 128 * (q + 1)], self.ident[:BL, :BL]
            )
            nc.scalar.copy(hT[:, BL * q : BL * (q + 1)], tp[:])
        return hT


def build_nc(TT=T, PSS=PS):
    nc = bacc.Bacc("TRN2", target_bir_lowering=False, debug=False,
                   num_devices=NCORES)

    # --- DRAM I/O (per core) ---
    d_x = nc.dram_tensor("x_img", [D + 1, TT * BL], MM_DT, kind="ExternalInput")
    d_rx0 = nc.dram_tensor("rx0", [D + 1, G], MM_DT, kind="ExternalInput")
    d_wh0 = nc.dram_tensor("wh0", [128, KH * G], MM_DT, kind="ExternalInput")
    d_w1 = nc.dram_tensor("w1", [128, 2 * KH * G], MM_DT, kind="ExternalInput")
    d_b1 = nc.dram_tensor("b1", [BL, G], F32, kind="ExternalInput")
    d_dw0 = nc.dram_tensor("dw0", [128, 2 * KH * G], MM_DT, kind="ExternalInput")
    d_db0 = nc.dram_tensor("db0", [BL, G], F32, kind="ExternalInput")
    d_dw1 = nc.dram_tensor("dw1", [128, 2 * KH * G], MM_DT, kind="ExternalInput")
    d_db1 = nc.dram_tensor("db1", [BL, G], F32, kind="ExternalInput")
    d_fcw = nc.dram_tensor("fcw", [128, KH * D], F32, kind="ExternalInput")
    d_fcb = nc.dram_tensor("fcb", [1, D], F32, kind="ExternalInput")
    d_hw = nc.dram_tensor("hw", [128, KH * (NCLS + 1)], F32, kind="ExternalInput")
    d_hb = nc.dram_tensor("hb", [1, NCLS + 1], F32, kind="ExternalInput")
    d_ones = nc.dram_tensor("ones", [1, BL], MM_DT, kind="ExternalInput")
    d_zt = nc.dram_tensor("zt", [128, KH * BL], MM_DT, kind="ExternalInput")

    d_fore = nc.dram_tensor("forecast", [BL, PSS * D], F32, kind="ExternalOutput")
    d_head = nc.dram_tensor("head", [BL, NCLS + 1], F32, kind="ExternalOutput")

    with tile.TileContext(nc) as tc:
        with (
            tc.tile_pool(name="const", bufs=1) as pc,
            tc.tile_pool(name="state", bufs=2) as ps,
            tc.tile_pool(name="psum_g", bufs=6, space="PSUM") as pg,
            tc.tile_pool(name="psum_t", bufs=2, space="PSUM") as pt,
        ):
            pools = {"ps": ps, "pg": pg, "pt": pt}

            ident = pc.tile([128, 128], F32)
            make_identity(nc, ident[:])
            ones16 = pc.tile([1, BL], MM_DT)
            nc.sync.dma_start(out=ones16[:], in_=d_ones[:])

            cell = _Cell(nc, pools, ident)

            # ---------------- encoder ----------------
            with tc.tile_pool(name="encw", bufs=1) as pe:
                xa = pe.tile([D + 1, TT * BL], MM_DT)
                nc.sync.dma_start(out=xa[:], in_=d_x[:])
                rx0 = pe.tile([D + 1, G], MM_DT)
                nc.sync.dma_start(out=rx0[:], in_=d_rx0[:])
                wh0 = pe.tile([128, KH * G], MM_DT)
                nc.sync.dma_start(out=wh0[:], in_=d_wh0[:])
                w1 = pe.tile([128, 2 * KH * G], MM_DT)
                nc.sync.dma_start(out=w1[:], in_=d_w1[:])
                b1 = pe.tile([BL, G], F32)
                nc.sync.dma_start(out=b1[:], in_=d_b1[:])

                wh0_ch = [wh0[:, G * k : G * (k + 1)] for k in range(KH)]
                w1_ch = [w1[:, G * k : G * (k + 1)] for k in range(2 * KH)]

                h0T = ps.tile([128, KH * BL], MM_DT, tag="hT_l0")
                nc.sync.dma_start(out=h0T[:], in_=d_zt[:])
                h1T = ps.tile([128, KH * BL], MM_DT, tag="hT_l1")
                nc.sync.dma_start(out=h1T[:], in_=d_zt[:])
                c0 = ps.tile([BL, H], F32, tag="c_l0")
                nc.vector.memset(c0[:], 0.0)
                c1 = ps.tile([BL, H], F32, tag="c_l1")
                nc.vector.memset(c1[:], 0.0)

                for t in range(TT):
                    # layer 0: gates = [x_t;1] @ rx0 + h0 @ Whh0.T
                    lhs0 = [xa[:, BL * t : BL * (t + 1)]] + [
                        h0T[:, BL * q : BL * (q + 1)] for q in range(KH)
                    ]
                    g0 = cell.gates(lhs0, [rx0[:, :]] + wh0_ch, "l0")
                    h0, c0 = cell.elementwise(g0, c0, None, "l0")
                    h0T = cell.transpose(h0, "l0")

                    # layer 1: gates = h0_t @ Wih1.T + h1 @ Whh1.T + b1
                    lhs1 = [h0T[:, BL * q : BL * (q + 1)] for q in range(KH)] + [
                        h1T[:, BL * q : BL * (q + 1)] for q in range(KH)
                    ]
                    g1 = cell.gates(lhs1, w1_ch, "l1")
                    h1, c1 = cell.elementwise(g1, c1, b1, "l1")
                    h1T = cell.transpose(h1, "l1")

            # ---------------- heads + decoder ----------------
            with tc.tile_pool(name="decw", bufs=1) as pd:
                dw0 = pd.tile([128, 2 * KH * G], MM_DT)
                nc.sync.dma_start(out=dw0[:], in_=d_dw0[:])
                db0 = pd.tile([BL, G], F32)
                nc.sync.dma_start(out=db0[:], in_=d_db0[:])
                dw1 = pd.tile([128, 2 * KH * G], MM_DT)
                nc.sync.dma_start(out=dw1[:], in_=d_dw1[:])
                db1 = pd.tile([BL, G], F32)
                nc.sync.dma_start(out=db1[:], in_=d_db1[:])
                fcw = pd.tile([128, KH * D], F32)
                nc.sync.dma_start(out=fcw[:], in_=d_fcw[:])
                fcb = pd.tile([1, D], F32)
                nc.sync.dma_start(out=fcb[:], in_=d_fcb[:])
                hw = pd.tile([128, KH * (NCLS + 1)], F32)
                nc.sync.dma_start(out=hw[:], in_=d_hw[:])
                hb = pd.tile([1, NCLS + 1], F32)
                nc.sync.dma_start(out=hb[:], in_=d_hb[:])

                dw0_ch = [dw0[:, G * k : G * (k + 1)] for k in range(2 * KH)]
                dw1_ch = [dw1[:, G * k : G * (k + 1)] for k in range(2 * KH)]

                # class/reg head: last @ [cls_W;reg_W].T + bias
                hp = pg.tile([BL, NCLS + 1], F32, tag="g")
                NH = NCLS + 1
                for q in range(KH):
                    _mm(nc, hp[:], h1T[:, BL * q : BL * (q + 1)].bitcast(F32),
                        hw[:, NH * q : NH * (q + 1)], start=(q == 0), stop=False)
                _mm(nc, hp[:], ones16[:].bitcast(F32), hb[:], start=False, stop=True)
                headsb = ps.tile([BL, NH], F32, tag="headsb")
                nc.scalar.copy(headsb[:], hp[:])
                nc.sync.dma_start(out=d_head[:], in_=headsb[:])

                fcout = pc.tile([BL, PSS * D], F32)
                inpT = h1T
                for s in range(PSS):
                    lhs0 = [inpT[:, BL * q : BL * (q + 1)] for q in range(KH)] + [
                        h0T[:, BL * q : BL * (q + 1)] for q in range(KH)
                    ]
                    g0 = cell.gates(lhs0, dw0_ch, "l0")
                    h0, c0 = cell.elementwise(g0, c0, db0, "l0")
                    h0T = cell.transpose(h0, "l0")

                    lhs1 = [h0T[:, BL * q : BL * (q + 1)] for q in range(KH)] + [
                        h1T[:, BL * q : BL * (q + 1)] for q in range(KH)
                    ]
                    g1 = cell.gates(lhs1, dw1_ch, "l1")
                    h1, c1 = cell.elementwise(g1, c1, db1, "l1")
                    h1T = cell.transpose(h1, "l1")
                    inpT = h1T

                    # pred = h1 @ fc_W.T + fc_b
                    pp = pg.tile([BL, D], F32, tag="g")
                    for q in range(KH):
                        _mm(nc, pp[:], h1T[:, BL * q : BL * (q + 1)].bitcast(F32),
                            fcw[:, D * q : D * (q + 1)], start=(q == 0), stop=False)
                    _mm(nc, pp[:], ones16[:].bitcast(F32), fcb[:], start=False, stop=True)
                    nc.scalar.copy(fcout[:, D * s : D * (s + 1)], pp[:])

                nc.sync.dma_start(out=d_fore[:], in_=fcout[:])

    nc.compile()
    return nc


def pack_inputs(inputs, TT=T):
    """Host-side packing of the full inputs into per-core input maps."""
    f = np.float32
    cat = np.concatenate

    def img(wT):  # [K, N] -> [128, (K/128)*N] chunk-packed image
        K, N = wT.shape
        assert K % 128 == 0
        return np.ascontiguousarray(
            wT.reshape(K // 128, 128, N).swapaxes(0, 1).reshape(128, -1)
        ).astype(f)

    rx0 = np.vstack(
        [inputs["enc_Wih0"].T, (inputs["enc_bih0"] + inputs["enc_bhh0"])[None, :]]
    ).astype(f)
    wh0 = img(inputs["enc_Whh0"].T)
    w1 = img(cat([inputs["enc_Wih1"].T, inputs["enc_Whh1"].T], axis=0))
    b1 = np.tile((inputs["enc_bih1"] + inputs["enc_bhh1"])[None, :], (BL, 1)).astype(f)
    dw0 = img(cat([inputs["dec_Wih0"].T, inputs["dec_Whh0"].T], axis=0))
    db0 = np.tile((inputs["dec_bih0"] + inputs["dec_bhh0"])[None, :], (BL, 1)).astype(f)
    dw1 = img(cat([inputs["dec_Wih1"].T, inputs["dec_Whh1"].T], axis=0))
    db1 = np.tile((inputs["dec_bih1"] + inputs["dec_bhh1"])[None, :], (BL, 1)).astype(f)
    fcw = img(inputs["fc_W"].T)
    fcb = inputs["fc_b"][None, :].astype(f)
    hwT = cat([inputs["cls_W"], inputs["reg_W"]], axis=0).T  # [512, 5]
    hw = img(hwT)
    hb = cat([inputs["cls_b"], inputs["reg_b"]])[None, :].astype(f)

    shared = dict(rx0=rx0, wh0=wh0, w1=w1, b1=b1, dw0=dw0, db0=db0,
                  dw1=dw1, db1=db1, fcw=fcw, fcb=fcb, hw=hw, hb=hb,
                  ones=np.ones((1, BL), f), zt=np.zeros((128, KH * BL), f))
    x = np.asarray(inputs["x"], dtype=f)
    in_maps = []
    for c in range(NCORES):
        xc = x[BL * c : BL * (c + 1), :TT, :]  # [16, TT, 6]
        x_img = np.empty((D + 1, TT * BL), f)
        x_img[:D] = xc.transpose(2, 1, 0).reshape(D, TT * BL)
        x_img[D] = 1.0
        in_maps.append(dict(shared, x_img=x_img))
    return in_maps


_NC_CACHE = {}


def _get_nc(TT=T, PSS=PS):
    key = (TT, PSS)
    if key not in _NC_CACHE:
        _NC_CACHE[key] = build_nc(TT, PSS)
    return _NC_CACHE[key]


def unpack_outputs(results, PSS=PS):
    fore = np.concatenate(
        [r["forecast"].reshape(BL, PSS, D) for r in results], axis=0
    )
    head = np.concatenate([r["head"] for r in results], axis=0)
    return fore, head[:, :NCLS], head[:, NCLS]


def kernel(**inputs):
    nc = _get_nc()
    in_maps = pack_inputs(inputs)
    res = run_bass_kernel_spmd(nc, in_maps, list(range(NCORES)))
    return unpack_outputs(res.results)


# revision 17
# speedup vs baseline: 3243.6341x; 3243.6341x over previous
"""Trainium2 Bass kernel for nn_MultiHeadHAR (2-layer LSTM encoder + 2-layer
LSTM decoder + heads), data-parallel over batch across 8 NeuronCores.

Self-contained: hardcodes shapes, packs weights on host into SBUF images,
builds one SPMD Bass program, runs it via run_bass_kernel_spmd, and
reassembles the full outputs.
"""

import numpy as np

import concourse.bacc as bacc
import concourse.bass as bass
import concourse.mybir as mybir
import concourse.tile as tile
from concourse.masks import make_identity
from concourse.bass_utils import run_bass_kernel_spmd

F32 = mybir.dt.float32
F32R = mybir.dt.float32r
AF = mybir.ActivationFunctionType

B, T, D, H, NCLS, PS = 128, 256, 6, 512, 4, 12
NCORES = 8
BL = B // NCORES          # 16 batch rows per core
G = 4 * H                 # 2048 gate width
KH = H // 128             # 4 partition chunks per hidden vector

# matmul input dtype (float32r streams 1 col/cycle vs 4 for float32)
MM_DT = F32R


def _mm(nc, out, lhsT, rhs, start, stop):
    nc.tensor.matmul(out, lhsT, rhs, start=start, stop=stop)


class _Cell:
    """Emits one LSTM cell step given transposed-state lhsT chunks."""

    def __init__(self, nc, pools, ident):
        self.nc = nc
        self.pools = pools
        self.ident = ident

    def gates(self, lhsT_chunks, w_img, tag):
        """lhsT_chunks: list of ([K,16] AP).  w_img: sbuf [128, nk*2048] AP
        (chunk k at cols [2048k:2048(k+1)]).  Returns 4 psum gate tiles."""
        nc, pg = self.nc, self.pools["pg"]
        gps = []
        for n in range(4):
            gp = pg.tile([BL, 512], F32, tag="g")
            nk = len(lhsT_chunks)
            for k, lhsT in enumerate(lhsT_chunks):
                rhs = w_img[k][:, 512 * n : 512 * (n + 1)]
                _mm(nc, gp[:], lhsT, rhs, start=(k == 0), stop=(k == nk - 1))
            gps.append(gp)
        return gps

    def elementwise(self, gps, c_prev, bias, tag):
        """gps: psum gate tiles [i,f,g,o]; bias: None or sbuf [16,2048] AP.
        Returns (h [16,512] sbuf, c_new [16,512] sbuf)."""
        nc, ps = self.nc, self.pools["ps"]
        acts = []
        for n, fn in enumerate([AF.Sigmoid, AF.Sigmoid, AF.Tanh, AF.Sigmoid]):
            a = ps.tile([BL, 512], F32, tag=f"a{n}_{tag}")
            if bias is not None:
                nc.vector.tensor_add(
                    gps[n][:], gps[n][:], bias[:, 512 * n : 512 * (n + 1)]
                )
            nc.scalar.activation(a[:], gps[n][:], fn)
            acts.append(a)
        i_s, f_s, g_t, o_s = acts
        # i_s <- i_s * tanh(g);  f_s <- f_s * c_prev;  c_new = i_s + f_s
        nc.vector.tensor_mul(i_s[:], i_s[:], g_t[:])
        nc.vector.tensor_mul(f_s[:], c_prev[:], f_s[:])
        c_new = ps.tile([BL, 512], F32, tag=f"c_{tag}")
        nc.vector.tensor_add(c_new[:], f_s[:], i_s[:])
        # g_t <- tanh(c_new);  h (in f_s slot) = o_s * g_t
        nc.scalar.activation(g_t[:], c_new[:], AF.Tanh)
        nc.vector.tensor_mul(f_s[:], o_s[:], g_t[:])
        return f_s, c_new

    def transpose(self, h, tag):
        """h: [16,512] sbuf -> hT sbuf tile [128, 64] (chunk q at cols 16q)."""
        nc = self.nc
        pt, ps = self.pools["pt"], self.pools["ps"]
        hT = ps.tile([128, KH * BL], MM_DT, tag=f"hT_{tag}")
        for q in range(KH):
            tp = pt.tile([128, BL], F32, tag="tp")
            nc.tensor.transpose(
                tp[:], h[:, 128 * q : 128 * (q + 1)], self.ident[:BL, :BL]
            )
            nc.scalar.copy(hT[:, BL * q : BL * (q + 1)], tp[:])
        return hT


def build_nc(TT=T, PSS=PS):
    nc = bacc.Bacc("TRN2", target_bir_lowering=False, debug=False,
                   num_devices=NCORES)

    # --- DRAM I/O (per core) ---
    d_x = nc.dram_tensor("x_img", [D + 1, TT * BL], MM_DT, kind="ExternalInput")
    d_rx0 = nc.dram_tensor("rx0", [D + 1, G], MM_DT, kind="ExternalInput")
    d_wh0 = nc.dram_tensor("wh0", [128, KH * G], MM_DT, kind="ExternalInput")
    d_w1 = nc.dram_tensor("w1", [128, 2 * KH * G], MM_DT, kind="ExternalInput")
    d_b1 = nc.dram_tensor("b1", [BL, G], F32, kind="ExternalInput")
    d_dw0 = nc.dram_tensor("dw0", [128, 2 * KH * G], MM_DT, kind="ExternalInput")
    d_db0 = nc.dram_tensor("db0", [BL, G], F32, kind="ExternalInput")
    d_dw1 = nc.dram_tensor("dw1", [128, 2 * KH * G], MM_DT, kind="ExternalInput")
    d_db1 = nc.dram_tensor("db1", [BL, G], F32, kind="ExternalInput")
    d_fcw = nc.dram_tensor("fcw", [128, KH * D], F32, kind="ExternalInput")
    d_fcb = nc.dram_tensor("fcb", [1, D], F32, kind="ExternalInput")
    d_hw = nc.dram_tensor("hw", [128, KH * (NCLS + 1)], F32, kind="ExternalInput")
    d_hb = nc.dram_tensor("hb", [1, NCLS + 1], F32, kind="ExternalInput")
    d_ones = nc.dram_tensor("ones", [1, BL], MM_DT, kind="ExternalInput")
    d_zt = nc.dram_tensor("zt", [128, KH * BL], MM_DT, kind="ExternalInput")

    d_fore = nc.dram_tensor("forecast", [BL, PSS * D], F32, kind="ExternalOutput")
    d_head = nc.dram_tensor("head", [BL, NCLS + 1], F32, kind="ExternalOutput")

    with tile.TileContext(nc) as tc:
        with (
            tc.tile_pool(name="const", bufs=1) as pc,
            tc.tile_pool(name="state", bufs=2) as ps,
            tc.tile_pool(name="psum_g", bufs=6, space="PSUM") as pg,
            tc.tile_pool(name="psum_t", bufs=2, space="PSUM") as pt,
        ):
            pools = {"ps": ps, "pg": pg, "pt": pt}

            ident = pc.tile([128, 128], F32)
            make_identity(nc, ident[:])
            ones16 = pc.tile([1, BL], MM_DT)
            nc.sync.dma_start(out=ones16[:], in_=d_ones[:])

            cell = _Cell(nc, pools, ident)

            # ---------------- encoder ----------------
            with tc.tile_pool(name="encw", bufs=1) as pe:
                xa = pe.tile([D + 1, TT * BL], MM_DT)
                nc.sync.dma_start(out=xa[:], in_=d_x[:])
                rx0 = pe.tile([D + 1, G], MM_DT)
                nc.sync.dma_start(out=rx0[:], in_=d_rx0[:])
                wh0 = pe.tile([128, KH * G], MM_DT)
                nc.sync.dma_start(out=wh0[:], in_=d_wh0[:])
                w1 = pe.tile([128, 2 * KH * G], MM_DT)
                nc.sync.dma_start(out=w1[:], in_=d_w1[:])
                b1 = pe.tile([BL, G], F32)
                nc.sync.dma_start(out=b1[:], in_=d_b1[:])

                wh0_ch = [wh0[:, G * k : G * (k + 1)] for k in range(KH)]
                w1_ch = [w1[:, G * k : G * (k + 1)] for k in range(2 * KH)]

                h0T = ps.tile([128, KH * BL], MM_DT, tag="hT_l0")
                nc.sync.dma_start(out=h0T[:], in_=d_zt[:])
                h1T = ps.tile([128, KH * BL], MM_DT, tag="hT_l1")
                nc.sync.dma_start(out=h1T[:], in_=d_zt[:])
                c0 = ps.tile([BL, H], F32, tag="c_l0")
                nc.vector.memset(c0[:], 0.0)
                c1 = ps.tile([BL, H], F32, tag="c_l1")
                nc.vector.memset(c1[:], 0.0)

                for t in range(TT):
                    # layer 0: gates = [x_t;1] @ rx0 + h0 @ Whh0.T
                    lhs0 = [xa[:, BL * t : BL * (t + 1)]] + [
                        h0T[:, BL * q : BL * (q + 1)] for q in range(KH)
                    ]
                    g0 = cell.gates(lhs0, [rx0[:, :]] + wh0_ch, "l0")
                    h0, c0 = cell.elementwise(g0, c0, None, "l0")
                    h0T = cell.transpose(h0, "l0")

                    # layer 1: gates = h0_t @ Wih1.T + h1 @ Whh1.T + b1
                    lhs1 = [h1T[:, BL * q : BL * (q + 1)] for q in range(KH)] + [
                        h0T[:, BL * q : BL * (q + 1)] for q in range(KH)
                    ]
                    g1 = cell.gates(lhs1, w1_ch, "l1")
                    h1, c1 = cell.elementwise(g1, c1, b1, "l1")
                    h1T = cell.transpose(h1, "l1")

            # ---------------- heads + decoder ----------------
            with tc.tile_pool(name="decw", bufs=1) as pd:
                dw0 = pd.tile([128, 2 * KH * G], MM_DT)
                nc.sync.dma_start(out=dw0[:], in_=d_dw0[:])
                db0 = pd.tile([BL, G], F32)
                nc.sync.dma_start(out=db0[:], in_=d_db0[:])
                dw1 = pd.tile([128, 2 * KH * G], MM_DT)
                nc.sync.dma_start(out=dw1[:], in_=d_dw1[:])
                db1 = pd.tile([BL, G], F32)
                nc.sync.dma_start(out=db1[:], in_=d_db1[:])
                fcw = pd.tile([128, KH * D], F32)
                nc.sync.dma_start(out=fcw[:], in_=d_fcw[:])
                fcb = pd.tile([1, D], F32)
                nc.sync.dma_start(out=fcb[:], in_=d_fcb[:])
                hw = pd.tile([128, KH * (NCLS + 1)], F32)
                nc.sync.dma_start(out=hw[:], in_=d_hw[:])
                hb = pd.tile([1, NCLS + 1], F32)
                nc.sync.dma_start(out=hb[:], in_=d_hb[:])

                dw0_ch = [dw0[:, G * k : G * (k + 1)] for k in range(2 * KH)]
                dw1_ch = [dw1[:, G * k : G * (k + 1)] for k in range(2 * KH)]

                # class/reg head: last @ [cls_W;reg_W].T + bias
                hp = pg.tile([BL, NCLS + 1], F32, tag="g")
                NH = NCLS + 1
                for q in range(KH):
                    _mm(nc, hp[:], h1T[:, BL * q : BL * (q + 1)].bitcast(F32),
                        hw[:, NH * q : NH * (q + 1)], start=(q == 0), stop=False)
                _mm(nc, hp[:], ones16[:].bitcast(F32), hb[:], start=False, stop=True)
                headsb = ps.tile([BL, NH], F32, tag="headsb")
                nc.scalar.copy(headsb[:], hp[:])
                nc.sync.dma_start(out=d_head[:], in_=headsb[:])

                fcout = pc.tile([BL, PSS * D], F32)
                inpT = h1T
                for s in range(PSS):
                    lhs0 = [h0T[:, BL * q : BL * (q + 1)] for q in range(KH)] + [
                        inpT[:, BL * q : BL * (q + 1)] for q in range(KH)
                    ]
                    g0 = cell.gates(lhs0, dw0_ch, "l0")
                    h0, c0 = cell.elementwise(g0, c0, db0, "l0")
                    h0T = cell.transpose(h0, "l0")

                    lhs1 = [h1T[:, BL * q : BL * (q + 1)] for q in range(KH)] + [
                        h0T[:, BL * q : BL * (q + 1)] for q in range(KH)
                    ]
                    g1 = cell.gates(lhs1, dw1_ch, "l1")
                    h1, c1 = cell.elementwise(g1, c1, db1, "l1")
                    h1T = cell.transpose(h1, "l1")
                    inpT = h1T

                    # pred = h1 @ fc_W.T + fc_b
                    pp = pg.tile([BL, D], F32, tag="g")
                    for q in range(KH):
                        _mm(nc, pp[:], h1T[:, BL * q : BL * (q + 1)].bitcast(F32),
                            fcw[:, D * q : D * (q + 1)], start=(q == 0), stop=False)
                    _mm(nc, pp[:], ones16[:].bitcast(F32), fcb[:], start=False, stop=True)
                    nc.scalar.copy(fcout[:, D * s : D * (s + 1)], pp[:])

                nc.sync.dma_start(out=d_fore[:], in_=fcout[:])

    nc.compile()
    return nc


def pack_inputs(inputs, TT=T):
    """Host-side packing of the full inputs into per-core input maps."""
    f = np.float32
    cat = np.concatenate

    def img(wT):  # [K, N] -> [128, (K/128)*N] chunk-packed image
        K, N = wT.shape
        assert K % 128 == 0
        return np.ascontiguousarray(
            wT.reshape(K // 128, 128, N).swapaxes(0, 1).reshape(128, -1)
        ).astype(f)

    rx0 = np.vstack(
        [inputs["enc_Wih0"].T, (inputs["enc_bih0"] + inputs["enc_bhh0"])[None, :]]
    ).astype(f)
    wh0 = img(inputs["enc_Whh0"].T)
    w1 = img(cat([inputs["enc_Whh1"].T, inputs["enc_Wih1"].T], axis=0))
    b1 = np.tile((inputs["enc_bih1"] + inputs["enc_bhh1"])[None, :], (BL, 1)).astype(f)
    dw0 = img(cat([inputs["dec_Whh0"].T, inputs["dec_Wih0"].T], axis=0))
    db0 = np.tile((inputs["dec_bih0"] + inputs["dec_bhh0"])[None, :], (BL, 1)).astype(f)
    dw1 = img(cat([inputs["dec_Whh1"].T, inputs["dec_Wih1"].T], axis=0))
    db1 = np.tile((inputs["dec_bih1"] + inputs["dec_bhh1"])[None, :], (BL, 1)).astype(f)
    fcw = img(inputs["fc_W"].T)
    fcb = inputs["fc_b"][None, :].astype(f)
    hwT = cat([inputs["cls_W"], inputs["reg_W"]], axis=0).T  # [512, 5]
    hw = img(hwT)
    hb = cat([inputs["cls_b"], inputs["reg_b"]])[None, :].astype(f)

    shared = dict(rx0=rx0, wh0=wh0, w1=w1, b1=b1, dw0=dw0, db0=db0,
                  dw1=dw1, db1=db1, fcw=fcw, fcb=fcb, hw=hw, hb=hb,
                  ones=np.ones((1, BL), f), zt=np.zeros((128, KH * BL), f))
    x = np.asarray(inputs["x"], dtype=f)
    in_maps = []
    for c in range(NCORES):
        xc = x[BL * c : BL * (c + 1), :TT, :]  # [16, TT, 6]
        x_img = np.empty((D + 1, TT * BL), f)
        x_img[:D] = xc.transpose(2, 1, 0).reshape(D, TT * BL)
        x_img[D] = 1.0
        in_maps.append(dict(shared, x_img=x_img))
    return in_maps


_NC_CACHE = {}


def _get_nc(TT=T, PSS=PS):
    key = (TT, PSS)
    if key not in _NC_CACHE:
        _NC_CACHE[key] = build_nc(TT, PSS)
    return _NC_CACHE[key]


def unpack_outputs(results, PSS=PS):
    fore = np.concatenate(
        [r["forecast"].reshape(BL, PSS, D) for r in results], axis=0
    )
    head = np.concatenate([r["head"] for r in results], axis=0)
    return fore, head[:, :NCLS], head[:, NCLS]


def kernel(**inputs):
    nc = _get_nc()
    in_maps = pack_inputs(inputs)
    res = run_bass_kernel_spmd(nc, in_maps, list(range(NCORES)))
    return unpack_outputs(res.results)
